# revision 53
# baseline (speedup 1.0000x reference)
"""Multi-head attention (B=2, T=2048, F=1024, H=16) on 8 trn2 NeuronCores.

Sharding: tensor-parallel over heads — 2 heads per core. Each core computes
Q^T/K^T/V^T projections for its head pair (column-sliced Wq/Wk/Wv), runs
attention, and a row-sliced output projection producing a partial (B,T,F)
output; the host sums the 8 partials and adds bo.

Layout: everything is computed transposed (Q^T, K^T, V^T, S^T = K Q^T,
ctx^T) so the only on-chip transposes are 16 cheap 128x128 PE transposes
per batch to build token-major V for the PV matmul. A ones-column appended
to V makes the softmax denominator fall out of the PV matmul for free;
normalization is deferred to after PV (it scales matmul columns linearly).

v2 schedule: window-phased softmax pipeline. For each 1024-token query
window, phase A issues both heads' score matmuls adjacently at PE row
positions 0/64 (row-tiled 64-contraction matmuls execute concurrently)
and the exps; the resulting expS tiles are staged in SBUF. Phase B
replays them through the PV matmuls into [65, 512] PSUM chunks, one
head at a time with both column-halves advancing per t2 step so the
staged tiles free progressively. Phase B of window w runs on the PE
while the scalar engine runs phase A of window w+1, with projection /
v1-transpose / output-projection work woven in as filler, so the kernel
tracks the scalar engine's exp roofline.
"""

import os
from collections import deque

import numpy as np

import concourse.mybir as mybir
import concourse.tile as tile
from concourse import bacc
from concourse.bass_utils import run_bass_kernel_spmd

B, T, F = 2, 2048, 1024
H, DK = 16, 64
NCORES = 8
HPC = H // NCORES          # heads per core
HD = HPC * DK              # 128 head dims per core
KT_ = F // 128             # 8 contraction tiles for projections
TW = 1024                  # t1 window (exp free-dim)
NW = T // TW               # 2 windows
NT2 = T // 128             # 16 t2 tiles
ES_BUFS = 44               # staged expS tiles (2 windows worth + slack)

f32 = mybir.dt.float32
f32r = mybir.dt.float32r
bf16 = mybir.dt.bfloat16
fp16 = mybir.dt.float16
EXP = mybir.ActivationFunctionType.Exp
MULT = mybir.AluOpType.mult

MODE = os.environ.get("MHA_MODE", "fp16")
# Only the sync HW DMA queue returns reliable data in this environment:
# gpsimd (SWDGE) corrupts wholesale, and scalar-queue DMAs corrupt a
# subset of transfers. Keep everything on nc.sync.
DMA_MIX = os.environ.get("MHA_DMA", "sync") == "mix"
# Adjacent same-t2 S matmuls at row positions 0/64 (concurrent row tiling)
# produce WRONG results on HW — keep heads sequential per t2 step.
PAIR = os.environ.get("MHA_PAIR", "0") == "1"


def build_nc(include_bias: bool, mode: str = MODE):
    mdt = {"bf16": bf16, "fp16": fp16}.get(mode, f32r)
    nc = bacc.Bacc("TRN2", target_bir_lowering=False)

    odt = f32 if mdt == f32r else mdt
    # host-pre-tiled inputs: [B, group, partition, k-chunk, 512 tokens] so a
    # single [128, KT_*512] DMA per group moves 8KB contiguous lines
    NG = T // 512
    xqT = nc.dram_tensor("xqT", [B, NG, 128, KT_, 512], mdt,
                         kind="ExternalInput")
    xkT = nc.dram_tensor("xkT", [B, NG, 128, KT_, 512], mdt,
                         kind="ExternalInput")
    xvT = nc.dram_tensor("xvT", [B, NG, 128, KT_, 512], mdt,
                         kind="ExternalInput")
    wq = nc.dram_tensor("wq", [F, HD], mdt, kind="ExternalInput")
    wk = nc.dram_tensor("wk", [F, HD], mdt, kind="ExternalInput")
    wv = nc.dram_tensor("wv", [F, HD], mdt, kind="ExternalInput")
    wo = nc.dram_tensor("wo", [HD, F], mdt, kind="ExternalInput")
    ident_in = nc.dram_tensor("ident", [128, 128], mdt, kind="ExternalInput")
    # sel[:, c*64:(c+1)*64] = e_c selector (row c ones) for the recip bcast
    sel_in = nc.dram_tensor("sel", [8, 8 * 64], mdt, kind="ExternalInput")
    identr_in = nc.dram_tensor("identr", [128, 128], f32, kind="ExternalInput")
    if include_bias:
        bq = nc.dram_tensor("bq", [1, HD], mdt, kind="ExternalInput")
        bk = nc.dram_tensor("bk", [1, HD], mdt, kind="ExternalInput")
        bv = nc.dram_tensor("bv", [1, HD], mdt, kind="ExternalInput")
    out = nc.dram_tensor("out", [B, T, F], odt, kind="ExternalOutput")

    with tile.TileContext(nc) as tc:
        with (
            tc.tile_pool(name="const", bufs=1) as cpool,
            tc.tile_pool(name="xs", bufs=5) as xpool,
            tc.tile_pool(name="work", bufs=1) as wpool,
            tc.tile_pool(name="psum", bufs=1, space="PSUM") as psum,
        ):
            # ---- constants / weights resident in SBUF ----
            wq_s = cpool.tile([128, KT_, HD], mdt, tag="wq")
            wk_s = cpool.tile([128, KT_, HD], mdt, tag="wk")
            wv_s = cpool.tile([128, KT_, HD], mdt, tag="wv")
            wo_s = cpool.tile([HD, F], mdt, tag="wo")
            ident = cpool.tile([128, 128], mdt, tag="ident")
            sel = cpool.tile([8, 8 * 64], mdt, tag="sel")
            identr = cpool.tile([128, 128], f32, tag="identr")
            nc.sync.dma_start(sel[:], sel_in[:])
            nc.sync.dma_start(identr[:], identr_in[:])
            nc.sync.dma_start(wq_s[:], wq.rearrange("(k p) m -> p k m", p=128))
            nc.sync.dma_start(wk_s[:], wk.rearrange("(k p) m -> p k m", p=128))
            nc.sync.dma_start(wv_s[:], wv.rearrange("(k p) m -> p k m", p=128))
            nc.sync.dma_start(wo_s[:], wo[:])
            nc.sync.dma_start(ident[:], ident_in[:])

            with nc.allow_low_precision(reason="matmul operand rounding"):
                # ones column pair for V1 (written into cols 64 and 129)
                onescol_f = wpool.tile([128, 2], f32, tag="c_f2")
                nc.vector.memset(onescol_f[:], 1.0)
                onescol = cpool.tile([128, 2], mdt, tag="onescol")
                nc.vector.tensor_copy(onescol[:], onescol_f[:])
                if include_bias:
                    bq_s = cpool.tile([1, HD], mdt, tag="bq")
                    bk_s = cpool.tile([1, HD], mdt, tag="bk")
                    bv_s = cpool.tile([1, HD], mdt, tag="bv")
                    nc.sync.dma_start(bq_s[:], bq[:])
                    nc.sync.dma_start(bk_s[:], bk[:])
                    nc.sync.dma_start(bv_s[:], bv[:])
                    onesrow_f = wpool.tile([1, 512], f32, tag="c_f3")
                    nc.vector.memset(onesrow_f[:], 1.0)
                    onesrow = cpool.tile([1, 512], mdt, tag="onesrow")
                    nc.vector.tensor_copy(onesrow[:], onesrow_f[:])

            # per-batch persistent tiles. K is stored as one zero-padded
            # [128, T] tile PER HEAD (rows hsl = K_h, other rows = 0) so
            # the score matmuls use a full 128-row contraction with NO
            # tile_position: the fp16 FWL weight path races with
            # tile_position row offsets on this HW (nondeterministic
            # corruption); full-row stationaries are the stable path.
            qt = {}; kt = {}; vt = {}; v1 = {}; ctxT = {}
            for b in range(B):
                qt[b] = wpool.tile([HD, T], mdt, tag="qt", bufs=2, name=f"qt{b}")
                kt[b] = [wpool.tile([128, T], mdt, tag="kt", bufs=4,
                                    name=f"kt{b}{h}") for h in range(HPC)]
                vt[b] = wpool.tile([HD, T], mdt, tag="vt", bufs=2, name=f"vt{b}")
                v1[b] = wpool.tile([128, NT2, 2 * 65], mdt, tag="v1", bufs=2,
                                   name=f"v1{b}")
                ctxT[b] = wpool.tile([HD, T], mdt, tag="ctxT", bufs=2,
                                     name=f"ctxT{b}")
            with nc.allow_low_precision(reason="zero padding"):
                for b in range(B):
                    nc.vector.memset(kt[b][0][64:128, :], 0.0)
                    nc.vector.memset(kt[b][1][0:64, :], 0.0)

            PROJ_MATS = {
                "q": (xqT, wq_s),
                "k": (xkT, wk_s),
                "v": (xvT, wv_s),
            }

            def gen_proj1(b, which):
                """One matrix projection for batch b. One DMA per
                512-token group delivers all 8 k-chunks as 8KB contiguous
                lines (max DMA line efficiency); one group is prefetched
                ahead. Yields at item boundaries so it can be interleaved
                into attention.
                NOTE: only the sync HW queue is reliable here — gpsimd
                (SWDGE) and scalar-queue DMAs return corrupted data."""
                xsrc, w_s = PROJ_MATS[which]
                dst = {"q": qt[b], "k": kt[b], "v": vt[b]}[which]
                xgs = {}

                def dma_group(n):
                    xg = xpool.tile([128, KT_, 512], mdt, tag="xg",
                                    name=f"xg{n}")
                    nc.sync.dma_start(xg[:], xsrc[b, n])
                    xgs[n] = xg

                dma_group(0)
                for n in range(NG):
                    if n + 1 < NG:
                        dma_group(n + 1)
                    yield
                    ps = psum.tile([128, 512], f32, tag="pa", bufs=2)
                    sl = slice(n * 512, (n + 1) * 512)
                    xg = xgs.pop(n)
                    for k in range(KT_):
                        nc.tensor.matmul(
                            ps[:], w_s[:, k, :], xg[:, k, :],
                            start=(k == 0),
                            stop=(k == KT_ - 1) and not include_bias,
                        )
                        if k == 3:
                            yield
                    if include_bias:
                        bsrc = {"q": bq_s, "k": bk_s, "v": bv_s}[which]
                        nc.tensor.matmul(ps[:], bsrc[:], onesrow[:],
                                         start=False, stop=True)
                    with nc.allow_low_precision(reason="rounding"):
                        if which == "k":
                            nc.vector.tensor_copy(dst[0][0:64, sl],
                                                  ps[0:64, :])
                            nc.vector.tensor_copy(dst[1][64:128, sl],
                                                  ps[64:128, :])
                        else:
                            nc.vector.tensor_copy(dst[:, sl], ps[:])
                    yield

            def gen_proj(b):
                for which in ("q", "k", "v"):
                    for _ in gen_proj1(b, which):
                        yield

            def gen_v1(b):
                """Token-major V (+ones cols) via PE transposes of V^T."""
                for tcid in range(NT2):
                    pt = psum.tile([128, 128], mdt, tag="pa", bufs=2)
                    tsl = slice(tcid * 128, (tcid + 1) * 128)
                    nc.tensor.transpose(pt[:], vt[b][:, tsl], ident[:])
                    with nc.allow_low_precision(reason="rounding"):
                        nc.vector.tensor_copy(v1[b][:, tcid, 0:64], pt[:, 0:64])
                        nc.vector.tensor_copy(v1[b][:, tcid, 65:129],
                                              pt[:, 64:128])
                        nc.vector.tensor_copy(v1[b][:, tcid, 64:130:65],
                                              onescol[:])
                    if tcid % 2 == 1:
                        yield

            def gen_oproj(b, lo, hi, tail=False):
                """Output projection token-chunks [lo, hi) for batch b.
                The final window's output DMAs ride the scalar queue,
                which is idle once the last exp has issued."""
                for tcid in range(lo, hi):
                    tsl = slice(tcid * 128, (tcid + 1) * 128)
                    ob = wpool.tile([128, F], odt, tag="ob", bufs=2)
                    for half in range(2):
                        po = psum.tile([128, 512], f32, tag="pa", bufs=2)
                        fsl = slice(half * 512, (half + 1) * 512)
                        nc.tensor.matmul(po[:], ctxT[b][:, tsl], wo_s[:, fsl],
                                         start=True, stop=True)
                        with nc.allow_low_precision(reason="rounding"):
                            nc.vector.tensor_copy(ob[:, fsl], po[:])
                    (nc.scalar if (DMA_MIX and tail) else nc.sync).dma_start(
                        out[b, tsl, :], ob[:])
                    yield

            pending = deque()

            def warm(nmm=3):
                """Dummy matmuls: keep the PE instruction streak alive so
                HAM never re-throttles the array to half clock."""
                pw = psum.tile([128, 128], f32, tag="pa", bufs=2)
                for _ in range(nmm):
                    nc.tensor.matmul(pw[:], ident[:], ident[:],
                                     start=True, stop=True)

            def consume():
                while pending:
                    try:
                        next(pending[0])
                        return
                    except StopIteration:
                        pending.popleft()
                warm()

            def norm_c(b, h, gsl, cts):
                """ctxT[hsl, gsl] = cts[0:64] / sums (sums = cts row 64).
                cts is a [65, 512] SBUF staging of the PV accumulator. The
                sums row is copied to partition 0 (aligned), transposed
                into columns with tiny PE transposes so the reciprocal
                runs across lanes, then transposed back and broadcast via
                selector matmuls."""
                nch = 4
                hsl = slice(h * 64, (h + 1) * 64)
                rc = wpool.tile([1, 512], f32, tag="rc", bufs=2)
                nc.vector.tensor_copy(rc[:], cts[64:65, :])
                pts = psum.tile([128, nch], f32, tag="pa", bufs=2)
                for c in range(nch):
                    nc.tensor.transpose(pts[:, c:c + 1],
                                        rc[0:1, c * 128:(c + 1) * 128],
                                        identr[0:1, 0:1])
                rcc = wpool.tile([128, nch], f32, tag="rcc", bufs=2)
                nc.vector.reciprocal(rcc[:], pts[:])
                consume()
                pr = psum.tile([nch, 128], f32, tag="pa", bufs=2)
                nc.tensor.transpose(pr[:], rcc[:], identr[:])
                rcr = wpool.tile([nch, 128], mdt, tag="rcr", bufs=2)
                with nc.allow_low_precision(reason="rounding"):
                    nc.vector.tensor_copy(rcr[:], pr[:])
                consume()
                scp = psum.tile([64, 512], f32, tag="ctx", bufs=2)
                for c in range(nch):
                    nc.tensor.matmul(scp[:, c * 128:(c + 1) * 128],
                                     sel[0:nch, c * 64:(c + 1) * 64], rcr[:],
                                     start=True, stop=True)
                sc = wpool.tile([64, 512], f32, tag="sc", bufs=2)
                nc.vector.tensor_copy(sc[:], scp[:])
                consume()
                with nc.allow_low_precision(reason="rounding"):
                    nc.vector.tensor_tensor(ctxT[b][hsl, gsl], cts[0:64, :],
                                            sc[:], MULT)

            def a_phase(b, n, store):
                """Scores + exp for window (b, n): both heads' 64-row score
                matmuls are emitted adjacently at row positions 0/64 so
                they execute concurrently in the PE array; expS tiles are
                staged into `store`."""
                for t2 in range(NT2):
                    t2sl = slice(t2 * 128, (t2 + 1) * 128)
                    s = [psum.tile([128, TW], f32, tag="st", bufs=2,
                                   name=f"s{h}")
                         for h in range(HPC)]
                    for h in range(HPC):
                        for q in range(TW // 512):
                            qsl = slice(n * TW + q * 512,
                                        n * TW + (q + 1) * 512)
                            nc.tensor.matmul(s[h][:, q * 512:(q + 1) * 512],
                                             kt[b][h][:, t2sl],
                                             qt[b][:, qsl],
                                             start=True, stop=True)
                    for h in range(HPC):
                        es = wpool.tile([128, TW], mdt, tag="es",
                                        bufs=ES_BUFS)
                        with nc.allow_low_precision(reason="rounding"):
                            nc.scalar.activation(es[:], s[h][:], EXP,
                                                 scale=0.125)
                        store[t2][h] = es
                    yield

            def b_phase(b, n, store):
                """PV + normalization for window (b, n) from staged expS.
                Per head, both 512-col ctx chunks advance each t2 step so
                each staged tile is fully consumed as soon as the step
                passes it (progressive pool recycling). The norms are
                batched after both heads' PV chains so the second head's
                staged tiles free early enough for the next window's exp
                allocations (avoids a pool-slot deadlock through the
                in-order PE queue)."""
                ctss = []
                for h in range(HPC):
                    vsl = slice(h * 65, (h + 1) * 65)
                    ctxa = psum.tile([65, 512], f32, tag="ctx", bufs=2)
                    ctxb = psum.tile([65, 512], f32, tag="ctx", bufs=2)
                    for t2 in range(NT2):
                        es = store[t2][h]
                        nc.tensor.matmul(ctxa[:], v1[b][:, t2, vsl],
                                         es[:, 0:512],
                                         start=(t2 == 0), stop=(t2 == NT2 - 1))
                        nc.tensor.matmul(ctxb[:], v1[b][:, t2, vsl],
                                         es[:, 512:1024],
                                         start=(t2 == 0), stop=(t2 == NT2 - 1))
                        if t2 % 4 == 3:
                            yield
                    for half, ctx in ((0, ctxa), (1, ctxb)):
                        cts = wpool.tile([65, 512], f32, tag="cts", bufs=4,
                                         name=f"cts{h}{half}")
                        nc.vector.tensor_copy(cts[:], ctx[:])
                        ctss.append((h, half, cts))
                    yield
                for h, half, cts in ctss:
                    norm_c(b, h,
                           slice(n * TW + half * 512,
                                 n * TW + (half + 1) * 512), cts)
                    yield
                tcw = T // 128 // NW
                pending.append(gen_oproj(b, n * tcw, (n + 1) * tcw,
                                         tail=(b == B - 1 and n == NW - 1)))

            # batch 0 Q/K projections run first, interleaved with each
            # other (nothing to hide behind); V projection and batch-1
            # work interleave into attention via the pending queue.
            gq, gk = gen_proj1(0, "q"), gen_proj1(0, "k")
            done = False
            while not done:
                done = True
                for g in (gq, gk):
                    try:
                        next(g)
                        done = False
                    except StopIteration:
                        pass
                # dummy matmuls ride the input-DMA wait: the PE streak
                # starts immediately so HAM is warm for the real work
                # small: just enough to bridge the (fast, pre-tiled)
                # input DMA wait without delaying the first real matmuls
                warm(2)
            pending.append(gen_proj1(0, "v"))
            pending.append(gen_v1(0))
            pending.append(gen_proj(1))
            pending.append(gen_v1(1))

            wins = [(b, n) for b in range(B) for n in range(NW)]
            prev_bg = None
            for b, n in wins:
                store = [[None] * HPC for _ in range(NT2)]
                for _ in a_phase(b, n, store):
                    if prev_bg is not None:
                        try:
                            next(prev_bg)
                        except StopIteration:
                            prev_bg = None
                    consume()
                # drain any leftover of the previous window's B phase
                if prev_bg is not None:
                    for _ in prev_bg:
                        consume()
                prev_bg = b_phase(b, n, store)
            # the scalar-paced final window leaves HAM at half clock; a
            # continuous dummy burst (>3us) un-throttles the PE before the
            # tail's dense PV/oproj drain
            warm(40)
            for _ in prev_bg:
                consume()
            while pending:
                try:
                    next(pending[0])
                except StopIteration:
                    pending.popleft()

    nc.compile()
    return nc


_CACHE = {}


def _get_nc(include_bias: bool):
    key = (include_bias, MODE)
    if key not in _CACHE:
        _CACHE[key] = build_nc(include_bias)
    return _CACHE[key]


def _reference_fallback(query, key_, value, mask, Wq, bq, Wk, bk, Wv, bv, Wo, bo):
    """Plain numpy fallback (only used if the mask is not all-ones)."""
    q = (query @ Wq + bq).reshape(B, T, H, DK).transpose(0, 2, 1, 3)
    k = (key_ @ Wk + bk).reshape(B, T, H, DK).transpose(0, 2, 1, 3)
    v = (value @ Wv + bv).reshape(B, T, H, DK).transpose(0, 2, 1, 3)
    scores = np.einsum("bhqd,bhkd->bhqk", q, k) / np.sqrt(np.float32(DK))
    scores = np.where(mask[:, None, :, :] > 0, scores,
                      np.float32(-10000.0)).astype(np.float32)
    scores -= scores.max(axis=-1, keepdims=True)
    e = np.exp(scores)
    attn = e / e.sum(axis=-1, keepdims=True)
    x = np.einsum("bhqk,bhkd->bhqd", attn, v)
    x = x.transpose(0, 2, 1, 3).reshape(B, T, F)
    return (x @ Wo + bo).astype(np.float32)


def _mdt_np(arr):
    if MODE == "bf16":
        import ml_dtypes
        return np.ascontiguousarray(arr).astype(ml_dtypes.bfloat16)
    if MODE == "fp16":
        return np.ascontiguousarray(arr).astype(np.float16)
    return np.ascontiguousarray(arr)


def _tile_x(x):
    """[B, T, F] -> [B, NG, 128, KT_, 512] host pre-tiling (see kernel)."""
    NG = T // 512
    xT = x.transpose(0, 2, 1)                       # [B, F, T]
    xT = xT.reshape(B, KT_, 128, NG, 512)
    return xT.transpose(0, 3, 2, 1, 4)


def make_in_maps(query, key_, value, Wq, Wk, Wv, Wo, bq=None, bk=None, bv=None):
    xqT = _mdt_np(_tile_x(query))
    xkT = _mdt_np(_tile_x(key_))
    xvT = _mdt_np(_tile_x(value))
    ident = _mdt_np(np.eye(128, dtype=np.float32))
    identr = np.eye(128, dtype=np.float32)
    sel = np.zeros((8, 8 * 64), np.float32)
    for c in range(8):
        sel[c, c * 64:(c + 1) * 64] = 1.0
    in_maps = []
    for c in range(NCORES):
        csl = slice(c * HD, (c + 1) * HD)
        m = {
            "xqT": xqT, "xkT": xkT, "xvT": xvT, "ident": ident,
            "sel": _mdt_np(sel), "identr": identr,
            "wq": _mdt_np(Wq[:, csl]),
            "wk": _mdt_np(Wk[:, csl]),
            "wv": _mdt_np(Wv[:, csl]),
            "wo": _mdt_np(Wo[csl, :]),
        }
        if bq is not None:
            m["bq"] = _mdt_np(bq[None, csl])
            m["bk"] = _mdt_np(bk[None, csl])
            m["bv"] = _mdt_np(bv[None, csl])
        in_maps.append(m)
    return in_maps


def kernel(**inputs) -> np.ndarray:
    query = np.asarray(inputs["query"], np.float32)
    key_ = np.asarray(inputs.get("key_", inputs.get("key")), np.float32)
    value = np.asarray(inputs["value"], np.float32)
    mask = np.asarray(inputs["mask"])
    Wq, bq = np.asarray(inputs["Wq"], np.float32), np.asarray(inputs["bq"], np.float32)
    Wk, bk = np.asarray(inputs["Wk"], np.float32), np.asarray(inputs["bk"], np.float32)
    Wv, bv = np.asarray(inputs["Wv"], np.float32), np.asarray(inputs["bv"], np.float32)
    Wo, bo = np.asarray(inputs["Wo"], np.float32), np.asarray(inputs["bo"], np.float32)

    if not (mask > 0).all():
        return _reference_fallback(query, key_, value, mask,
                                   Wq, bq, Wk, bk, Wv, bv, Wo, bo)

    include_bias = bool(np.any(bq) or np.any(bk) or np.any(bv))
    nc = _get_nc(include_bias)
    if include_bias:
        in_maps = make_in_maps(query, key_, value, Wq, Wk, Wv, Wo, bq, bk, bv)
    else:
        in_maps = make_in_maps(query, key_, value, Wq, Wk, Wv, Wo)

    # The very first NEFF execution after a load can read tiles before
    # their producers land (first-run corruption observed on this HW);
    # subsequent executions are deterministic. Run once to warm, then
    # take the second execution's result.
    run_bass_kernel_spmd(nc, in_maps, core_ids=list(range(NCORES)))
    res = run_bass_kernel_spmd(nc, in_maps, core_ids=list(range(NCORES)))
    total = np.asarray(res.results[0]["out"], np.float32)
    for c in range(1, NCORES):
        total = total + np.asarray(res.results[c]["out"], np.float32)
    return (total + bo).astype(np.float32)
